# revision 9
# baseline (speedup 1.0000x reference)
"""Trainium2 Bass kernel for nn_CLS_1889785610440.

Pipeline (per reference.py):
  3 scalar Elman RNNs over T in {4,8,16} for N=B*M*E lanes -> last hidden
  -> 1x3 conv over scales -> scalar RNN over M=64 -> BatchNorm1d (batch
  stats) -> ReLU -> Linear(E,C) -> softmax.

Sharding: data-parallel over the batch dim B=128 -> 16 samples per core
(contiguous N/8 lane chunks of a0/a1/a2). Only the BatchNorm statistics
cross cores (one 2KB AllReduce).

Device mapping:
  - stage-1 recurrence step:  psum = diag(wih_s) @ x_t + diag(whh_s) @ h
    on TensorE (two accumulating matmuls per step, 128x512 tiles), then
    h = tanh(psum + b) on ScalarE.  VectorE stays free.
  - conv: 3 accumulating diag matmuls; ScalarE copy folds rnn2's input
    scale/bias so the result is directly rnn2's per-step input u2.
  - rnn2: PE-transpose 128x128 blocks so lanes=(b_loc,e) sit on
    partitions (128 x 32 tile); per step one fused DVE
    scalar_tensor_tensor (h*whh2 + u2_m) + ScalarE tanh.  When
    |whh2| < 1 the recurrence is truncated to K steps with
    |whh2|^K < 1e-9 (only the last hidden state is needed).
  - BN: per-core sum/sumsq -> AllReduce(128x4) -> mean/var; inv_std via
    exp(-0.5*ln(var+eps)) (stays in the ln/exp ACT table set);
    normalize+relu on DVE; FC via two matmuls; softmax on-device.
"""

import numpy as np

import concourse.bacc as bacc
import concourse.tile as tile
import concourse.mybir as mybir
from concourse.bass_utils import run_bass_kernel_spmd

# Problem constants (hardcoded per spec).
B = 128
E = 256
M = 64
S = 3
C = 5
SCALES = [4, 8, 16]
EPS = 1e-5

N_CORES = 8
N = B * M * E              # 2097152 lanes
N8 = N // N_CORES          # 262144 lanes per core
F = 512                    # free dim of a stage-1 tile
NCHUNK = N8 // (128 * F)   # 4 chunks of (128, 512) lanes per core
BLOC = B // N_CORES        # 16 samples per core
L2 = BLOC * 2              # 32 rnn2 lanes per partition

FP32 = mybir.dt.float32
AF = mybir.ActivationFunctionType
ALU = mybir.AluOpType


def _build(params):
    """Build the Bass program. `params` holds host-side python floats and
    small numpy arrays derived from the model parameters."""
    nc = bacc.Bacc("TRN2", target_bir_lowering=False, debug=False,
                   enable_asserts=True, num_devices=N_CORES)

    a_dram = [
        nc.dram_tensor(f"a{i}", [N8 * T], FP32, kind="ExternalInput")
        for i, T in enumerate(SCALES)
    ]
    out_dram = nc.dram_tensor("out", [BLOC, C], FP32, kind="ExternalOutput")

    # Inline constants (baked into the NEFF, replicated on every core).
    # diag blocks: [wih0..2 | whh0..2 | cw0..2 | identity] as 128x128 fp32.
    eye = np.eye(128, dtype=np.float32)
    diag_blocks = (
        [eye * params["wih"][s] for s in range(S)]
        + [eye * params["whh"][s] for s in range(S)]
        + [eye * params["cw"][s] for s in range(S)]
        + [eye]
    )
    diag_np = np.concatenate(diag_blocks, axis=1)  # (128, 128*10)
    diag_c = nc.inline_tensor(diag_np, name="diagc")

    # FC weights packed for contraction over e_lo: W[e_lo, eh*C + c] =
    # fnn_w[c, eh*128 + e_lo]
    fw = params["fnn_w"]  # (C, E)
    wpack_np = np.concatenate(
        [fw[:, :128].T.astype(np.float32), fw[:, 128:].T.astype(np.float32)],
        axis=1)  # (128, 2C)
    wpack_c = nc.inline_tensor(wpack_np, name="wpack")

    # gamma/beta arranged (e_lo, e_hi):  [gamma | beta] -> (128, 4)
    g = params["gamma"].reshape(2, 128).T.astype(np.float32)
    bta = params["beta"].reshape(2, 128).T.astype(np.float32)
    gb_c = nc.inline_tensor(np.concatenate([g, bta], axis=1), name="gb")

    fnnb_c = nc.inline_tensor(
        params["fnn_b"].reshape(C, 1).astype(np.float32), name="fnnb")

    wih2 = params["wih2"]
    whh2 = params["whh2"]
    bias2 = wih2 * params["cb"] + params["bb2"]

    # activation biases as per-partition columns: [bb0, bb1, bb2, bias2, EPS]
    bias_np = np.tile(
        np.array([params["bb"][0], params["bb"][1], params["bb"][2],
                  bias2, EPS], np.float32)[None, :], (128, 1))
    bias_c = nc.inline_tensor(bias_np, name="biasc")

    # rnn2 truncation: error of last hidden <= |whh2|^K
    aw = abs(whh2)
    if aw < 1e-12:
        K = 1
    elif aw >= 1.0:
        K = M
    else:
        K = min(M, max(1, int(np.ceil(np.log(1e-9) / np.log(aw)))))

    from contextlib import ExitStack
    with tile.TileContext(nc) as tc, ExitStack() as ctx:
        singles = ctx.enter_context(tc.tile_pool(name="singles", bufs=1))
        xp = [ctx.enter_context(tc.tile_pool(name=f"x{s}", bufs=2))
              for s in range(S)]
        hp = ctx.enter_context(tc.tile_pool(name="h", bufs=4))
        hfp = ctx.enter_context(tc.tile_pool(name="hf", bufs=2))
        cvp = ctx.enter_context(tc.tile_pool(name="cv", bufs=2))
        r2p = ctx.enter_context(tc.tile_pool(name="r2", bufs=1))
        smp = ctx.enter_context(tc.tile_pool(name="sm", bufs=2))
        ps1 = ctx.enter_context(tc.tile_pool(name="ps1", bufs=4, space="PSUM"))
        psc = ctx.enter_context(tc.tile_pool(name="psc", bufs=1, space="PSUM"))
        pst = ctx.enter_context(tc.tile_pool(name="pst", bufs=1, space="PSUM"))
        psf = ctx.enter_context(tc.tile_pool(name="psf", bufs=1, space="PSUM"))
        dram = ctx.enter_context(tc.tile_pool(name="dram", bufs=1, space="DRAM"))

        diag_sb = singles.tile([128, 128 * 10], FP32)
        nc.sync.dma_start(out=diag_sb[:], in_=diag_c[:])
        wpack_sb = singles.tile([128, 2 * C], FP32)
        nc.sync.dma_start(out=wpack_sb[:], in_=wpack_c[:])
        gb_sb = singles.tile([128, 4], FP32)
        nc.sync.dma_start(out=gb_sb[:], in_=gb_c[:])
        fnnb_sb = singles.tile([C, 1], FP32)
        nc.sync.dma_start(out=fnnb_sb[:], in_=fnnb_c[:])
        bias_sb = singles.tile([128, 5], FP32)
        nc.sync.dma_start(out=bias_sb[:], in_=bias_c[:])

        def dwih(s):
            return diag_sb[:, s * 128:(s + 1) * 128]

        def dwhh(s):
            return diag_sb[:, (S + s) * 128:(S + s + 1) * 128]

        def dcw(s):
            return diag_sb[:, (2 * S + s) * 128:(2 * S + s + 1) * 128]

        ident = diag_sb[:, 3 * S * 128:(3 * S + 1) * 128]

        # rnn2 input u2, layout [e_lo, m, l] with l = b_loc*2 + e_hi
        rnn2buf = r2p.tile([128, M, L2], FP32)

        a_view = [
            a_dram[s].ap().rearrange("(c p ft) -> c p ft", c=NCHUNK, p=128)
            for s in range(S)
        ]

        for c in range(NCHUNK):
            xt = []
            for s, T in enumerate(SCALES):
                x = xp[s].tile([128, F, T], FP32)
                nc.sync.dma_start(
                    out=x[:].rearrange("p f t -> p (f t)"), in_=a_view[s][c])
                xt.append(x)

            # interleave the three scales' recurrences step by step
            h_cur = [None] * S
            hfin = []
            for s in range(S):
                hfin.append(hfp.tile([128, F], FP32, tag=f"hf{s}",
                                     name=f"hf{s}"))
            for t in range(max(SCALES)):
                for s, T in enumerate(SCALES):
                    if t >= T:
                        continue
                    ps = ps1.tile([128, F], FP32)
                    if t == 0:
                        nc.tensor.matmul(ps[:], dwih(s), xt[s][:, :, t],
                                         start=True, stop=True)
                    else:
                        nc.tensor.matmul(ps[:], dwih(s), xt[s][:, :, t],
                                         start=True, stop=False)
                        nc.tensor.matmul(ps[:], dwhh(s), h_cur[s][:],
                                         start=False, stop=True)
                    hn = hfin[s] if t == T - 1 else hp.tile(
                        [128, F], FP32, tag=f"h{s}")
                    nc.scalar.activation(hn[:], ps[:], AF.Tanh,
                                         bias=bias_sb[:, s:s + 1])
                    h_cur[s] = hn

            # conv over scales + fold rnn2 input affine:
            #   u2 = wih2*(sum_s cw_s*h_s + cb) + bih2 + bhh2
            pc = psc.tile([128, F], FP32)
            nc.tensor.matmul(pc[:], dcw(0), hfin[0][:], start=True, stop=False)
            nc.tensor.matmul(pc[:], dcw(1), hfin[1][:], start=False, stop=False)
            nc.tensor.matmul(pc[:], dcw(2), hfin[2][:], start=False, stop=True)
            cv = cvp.tile([128, F], FP32)
            nc.scalar.activation(cv[:], pc[:], AF.Identity,
                                 bias=bias_sb[:, 3:4], scale=wih2)

            # transpose each 128x128 block; scatter into rnn2buf
            for j in range(4):
                m_lo, e_hi = j // 2, j % 2
                pt = pst.tile([128, 128], FP32)
                nc.tensor.transpose(pt[:], cv[:, j * 128:(j + 1) * 128], ident)
                src = pt[:].rearrange("p (b v) -> p v b", b=4)
                dst = rnn2buf[:, m_lo::2, 8 * c + e_hi:8 * c + 8:2]
                nc.vector.tensor_copy(dst, src)

        # ---- rnn2 over m (truncated to last K steps) ----
        feat = smp.tile([128, L2], FP32, tag="feat")
        h2 = None
        for m in range(M - K, M):
            last = m == M - 1
            dst = feat if last else smp.tile([128, L2], FP32, tag="h2")
            if h2 is None:
                nc.scalar.activation(dst[:], rnn2buf[:, m, :], AF.Tanh)
            else:
                st = smp.tile([128, L2], FP32, tag="st")
                nc.vector.scalar_tensor_tensor(
                    st[:], h2[:], whh2, rnn2buf[:, m, :],
                    op0=ALU.mult, op1=ALU.add)
                nc.scalar.activation(dst[:], st[:], AF.Tanh)
            h2 = dst

        # ---- BatchNorm stats (partial) + AllReduce ----
        featsq = smp.tile([128, L2], FP32, tag="fsq")
        nc.vector.tensor_tensor(featsq[:], feat[:], feat[:], ALU.mult)
        stats = smp.tile([128, 4], FP32, tag="stats")
        fv = feat[:].rearrange("p (b eh) -> p eh b", b=BLOC)
        fsv = featsq[:].rearrange("p (b eh) -> p eh b", b=BLOC)
        nc.vector.tensor_reduce(stats[:, 0:2], fv, axis=mybir.AxisListType.X,
                                op=ALU.add)
        nc.vector.tensor_reduce(stats[:, 2:4], fsv, axis=mybir.AxisListType.X,
                                op=ALU.add)
        bin_ = dram.tile([128, 4], FP32, tag="bin")
        bout = dram.tile([128, 4], FP32, tag="bout")
        nc.gpsimd.dma_start(bin_[:], stats[:])
        nc.gpsimd.collective_compute(
            "AllReduce", ALU.add,
            replica_groups=[list(range(N_CORES))],
            ins=[bin_.opt()], outs=[bout.opt()])
        stg = smp.tile([128, 4], FP32, tag="stg")
        nc.gpsimd.dma_start(stg[:], bout[:])

        # mean/var/scale/shift (all (128,2): per (e_lo, e_hi))
        mean = smp.tile([128, 2], FP32, tag="mean")
        nc.vector.tensor_scalar(mean[:], stg[:, 0:2], 1.0 / B, None, ALU.mult)
        ex2 = smp.tile([128, 2], FP32, tag="ex2")
        nc.vector.tensor_scalar(ex2[:], stg[:, 2:4], 1.0 / B, None, ALU.mult)
        var = smp.tile([128, 2], FP32, tag="var")
        nc.vector.tensor_tensor(var[:], mean[:], mean[:], ALU.mult)
        nc.vector.tensor_tensor(var[:], ex2[:], var[:], ALU.subtract)
        lnv = smp.tile([128, 2], FP32, tag="lnv")
        nc.scalar.activation(lnv[:], var[:], AF.Ln, bias=bias_sb[:, 4:5])
        istd = smp.tile([128, 2], FP32, tag="istd")
        nc.scalar.activation(istd[:], lnv[:], AF.Exp, scale=-0.5)
        scl = smp.tile([128, 2], FP32, tag="scl")
        nc.vector.tensor_tensor(scl[:], istd[:], gb_sb[:, 0:2], ALU.mult)
        shf = smp.tile([128, 2], FP32, tag="shf")
        nc.vector.tensor_tensor(shf[:], mean[:], scl[:], ALU.mult)
        nc.vector.tensor_tensor(shf[:], gb_sb[:, 2:4], shf[:], ALU.subtract)

        # normalize + relu
        r = smp.tile([128, L2], FP32, tag="r")
        f3 = feat[:].rearrange("p (b eh) -> p b eh", b=BLOC)
        r3 = r[:].rearrange("p (b eh) -> p b eh", b=BLOC)
        for eh in range(2):
            nc.vector.tensor_scalar(
                r3[:, :, eh], f3[:, :, eh],
                scl[:, eh:eh + 1], shf[:, eh:eh + 1],
                op0=ALU.mult, op1=ALU.add)
        nc.vector.tensor_scalar_max(r[:], r[:], 0.0)

        # FC: logits^T (C, BLOC) = sum_eh Wpack_eh.T @ r[:, :, eh]
        pl = psf.tile([C, BLOC], FP32, tag="pl")
        nc.tensor.matmul(pl[:], wpack_sb[:, 0:C], r3[:, :, 0],
                         start=True, stop=False)
        nc.tensor.matmul(pl[:], wpack_sb[:, C:2 * C], r3[:, :, 1],
                         start=False, stop=True)
        lt = smp.tile([C, BLOC], FP32, tag="lt")
        nc.vector.tensor_scalar(lt[:], pl[:], fnnb_sb[:, 0:1], None, ALU.add)

        # transpose to (BLOC, C) and softmax along free dim
        pt2 = psf.tile([BLOC, C], FP32, tag="pt2")
        nc.tensor.transpose(pt2[:], lt[:], ident[0:C, 0:C])
        nmax = smp.tile([BLOC, 1], FP32, tag="nmax")
        nc.vector.tensor_reduce(nmax[:], pt2[:], axis=mybir.AxisListType.X,
                                op=ALU.max, negate=True)
        esb = smp.tile([BLOC, C], FP32, tag="esb")
        nc.scalar.activation(esb[:], pt2[:], AF.Exp, bias=nmax[:, 0:1])
        ssum = smp.tile([BLOC, 1], FP32, tag="ssum")
        nc.vector.tensor_reduce(ssum[:], esb[:], axis=mybir.AxisListType.X,
                                op=ALU.add)
        rin = smp.tile([BLOC, 1], FP32, tag="rin")
        nc.vector.reciprocal(rin[:], ssum[:])
        osb = smp.tile([BLOC, C], FP32, tag="osb")
        nc.vector.tensor_scalar(osb[:], esb[:], rin[:, 0:1], None, ALU.mult)
        nc.sync.dma_start(out=out_dram[:], in_=osb[:])

    nc.compile()
    return nc


def kernel(a0, a1, a2, rnn1_wih, rnn1_whh, rnn1_bih, rnn1_bhh,
           conv_w, conv_b, rnn2_wih, rnn2_whh, rnn2_bih, rnn2_bhh,
           norm_gamma, norm_beta, fnn_w, fnn_b, _bench=None):
    params = {
        "wih": [float(rnn1_wih[s]) for s in range(S)],
        "whh": [float(rnn1_whh[s]) for s in range(S)],
        "bb": [float(rnn1_bih[s]) + float(rnn1_bhh[s]) for s in range(S)],
        "cw": [float(conv_w[s]) for s in range(S)],
        "cb": float(conv_b[0]),
        "wih2": float(rnn2_wih[0]),
        "whh2": float(rnn2_whh[0]),
        "bb2": float(rnn2_bih[0]) + float(rnn2_bhh[0]),
        "gamma": np.asarray(norm_gamma, np.float32),
        "beta": np.asarray(norm_beta, np.float32),
        "fnn_w": np.asarray(fnn_w, np.float32),
        "fnn_b": np.asarray(fnn_b, np.float32),
    }
    nc = _build(params)

    flat = [np.ascontiguousarray(np.asarray(a, np.float32)).reshape(-1)
            for a in (a0, a1, a2)]
    in_maps = []
    for k in range(N_CORES):
        m = {}
        for i, T in enumerate(SCALES):
            sz = N8 * T
            m[f"a{i}"] = flat[i][k * sz:(k + 1) * sz]
        in_maps.append(m)

    kw = dict(_bench) if _bench else {}
    res = run_bass_kernel_spmd(nc, in_maps, core_ids=list(range(N_CORES)),
                               **kw)
    out = np.concatenate([res.results[k]["out"] for k in range(N_CORES)],
                         axis=0)
    if _bench is not None:
        kernel.last_result = res
    return out


# revision 12
# speedup vs baseline: 1.7112x; 1.7112x over previous
"""Trainium2 Bass kernel for nn_CLS_1889785610440.

Pipeline (per reference.py):
  3 scalar Elman RNNs over T in {4,8,16} for N=B*M*E lanes -> last hidden
  -> 1x3 conv over scales -> scalar RNN over M=64 -> BatchNorm1d (batch
  stats) -> ReLU -> Linear(E,C) -> softmax.

Sharding: data-parallel over the batch dim B=128 -> 16 samples per core
(contiguous N/8 lane chunks of a0/a1/a2). Only the BatchNorm statistics
cross cores (one 2KB AllReduce).

Device mapping:
  - stage-1 recurrence step:  psum = diag(wih_s) @ x_t + diag(whh_s) @ h
    on TensorE (two accumulating matmuls per step, 128x512 tiles), then
    h = tanh(psum + b) on ScalarE.  VectorE stays free.
  - conv: 3 accumulating diag matmuls; ScalarE copy folds rnn2's input
    scale/bias so the result is directly rnn2's per-step input u2.
  - rnn2: PE-transpose 128x128 blocks so lanes=(b_loc,e) sit on
    partitions (128 x 32 tile); per step one fused DVE
    scalar_tensor_tensor (h*whh2 + u2_m) + ScalarE tanh.  When
    |whh2| < 1 the recurrence is truncated to K steps with
    |whh2|^K < 1e-9 (only the last hidden state is needed).
  - BN: per-core sum/sumsq -> AllReduce(128x4) -> mean/var; inv_std via
    exp(-0.5*ln(var+eps)) (stays in the ln/exp ACT table set);
    normalize+relu on DVE; FC via two matmuls; softmax on-device.
"""

import numpy as np

import concourse.bacc as bacc
import concourse.tile as tile
import concourse.mybir as mybir
from concourse.bass_utils import run_bass_kernel_spmd

# Problem constants (hardcoded per spec).
B = 128
E = 256
M = 64
S = 3
C = 5
SCALES = [4, 8, 16]
EPS = 1e-5

N_CORES = 8
N = B * M * E              # 2097152 lanes
N8 = N // N_CORES          # 262144 lanes per core
F = 512                    # free dim of a stage-1 tile
NCHUNK = N8 // (128 * F)   # 4 chunks of (128, 512) lanes per core
BLOC = B // N_CORES        # 16 samples per core
L2 = BLOC * 2              # 32 rnn2 lanes per partition

FP32 = mybir.dt.float32
AF = mybir.ActivationFunctionType
ALU = mybir.AluOpType


def _build(params, repeat=1):
    """Build the Bass program. `params` holds host-side python floats and
    small numpy arrays derived from the model parameters.  `repeat` re-emits
    the whole compute body K times (benchmarking only — differential timing
    against repeat=1 cancels the host<->device transfer baseline)."""
    nc = bacc.Bacc("TRN2", target_bir_lowering=False, debug=False,
                   enable_asserts=True, num_devices=N_CORES)

    a_dram = [
        nc.dram_tensor(f"a{i}", [N8 * T], FP32, kind="ExternalInput")
        for i, T in enumerate(SCALES)
    ]
    out_dram = nc.dram_tensor("out", [BLOC, C], FP32, kind="ExternalOutput")

    # Inline constants (baked into the NEFF, replicated on every core).
    # diag blocks: [wih0..2 | whh0..2 | cw0..2 | identity] as 128x128 fp32.
    eye = np.eye(128, dtype=np.float32)
    diag_blocks = (
        [eye * params["wih"][s] for s in range(S)]
        + [eye * params["whh"][s] for s in range(S)]
        + [eye * params["cw"][s] for s in range(S)]
        + [eye]
    )
    diag_np = np.concatenate(diag_blocks, axis=1)  # (128, 128*10)
    diag_c = nc.inline_tensor(diag_np, name="diagc")

    # FC weights packed for contraction over e_lo: W[e_lo, eh*C + c] =
    # fnn_w[c, eh*128 + e_lo]
    fw = params["fnn_w"]  # (C, E)
    wpack_np = np.concatenate(
        [fw[:, :128].T.astype(np.float32), fw[:, 128:].T.astype(np.float32)],
        axis=1)  # (128, 2C)
    wpack_c = nc.inline_tensor(wpack_np, name="wpack")

    # gamma/beta arranged (e_lo, e_hi):  [gamma | beta] -> (128, 4)
    g = params["gamma"].reshape(2, 128).T.astype(np.float32)
    bta = params["beta"].reshape(2, 128).T.astype(np.float32)
    gb_c = nc.inline_tensor(np.concatenate([g, bta], axis=1), name="gb")

    fnnb_c = nc.inline_tensor(
        params["fnn_b"].reshape(C, 1).astype(np.float32), name="fnnb")

    wih2 = params["wih2"]
    whh2 = params["whh2"]
    bias2 = wih2 * params["cb"] + params["bb2"]

    # activation biases as per-partition columns: [bb0, bb1, bb2, bias2, EPS]
    bias_np = np.tile(
        np.array([params["bb"][0], params["bb"][1], params["bb"][2],
                  bias2, EPS], np.float32)[None, :], (128, 1))
    bias_c = nc.inline_tensor(bias_np, name="biasc")

    # rnn2 truncation: error of last hidden <= |whh2|^K
    aw = abs(whh2)
    if aw < 1e-12:
        K = 1
    elif aw >= 1.0:
        K = M
    else:
        K = min(M, max(1, int(np.ceil(np.log(1e-9) / np.log(aw)))))

    from contextlib import ExitStack
    with tile.TileContext(nc) as tc, ExitStack() as ctx:
        singles = ctx.enter_context(tc.tile_pool(name="singles", bufs=1))
        xp = [ctx.enter_context(tc.tile_pool(name=f"x{s}", bufs=2))
              for s in range(S)]
        hp = ctx.enter_context(tc.tile_pool(name="h", bufs=4))
        hfp = ctx.enter_context(tc.tile_pool(name="hf", bufs=2))
        cvp = ctx.enter_context(tc.tile_pool(name="cv", bufs=2))
        r2p = ctx.enter_context(tc.tile_pool(name="r2", bufs=1))
        smp = ctx.enter_context(tc.tile_pool(name="sm", bufs=2))
        ps1 = ctx.enter_context(tc.tile_pool(name="ps1", bufs=4, space="PSUM"))
        psc = ctx.enter_context(tc.tile_pool(name="psc", bufs=1, space="PSUM"))
        pst = ctx.enter_context(tc.tile_pool(name="pst", bufs=1, space="PSUM"))
        psf = ctx.enter_context(tc.tile_pool(name="psf", bufs=1, space="PSUM"))
        dram = ctx.enter_context(tc.tile_pool(name="dram", bufs=1, space="DRAM"))

        diag_sb = singles.tile([128, 128 * 10], FP32)
        nc.sync.dma_start(out=diag_sb[:], in_=diag_c[:])
        wpack_sb = singles.tile([128, 2 * C], FP32)
        nc.sync.dma_start(out=wpack_sb[:], in_=wpack_c[:])
        gb_sb = singles.tile([128, 4], FP32)
        nc.sync.dma_start(out=gb_sb[:], in_=gb_c[:])
        fnnb_sb = singles.tile([C, 1], FP32)
        nc.sync.dma_start(out=fnnb_sb[:], in_=fnnb_c[:])
        bias_sb = singles.tile([128, 5], FP32)
        nc.sync.dma_start(out=bias_sb[:], in_=bias_c[:])

        def dwih(s):
            return diag_sb[:, s * 128:(s + 1) * 128]

        def dwhh(s):
            return diag_sb[:, (S + s) * 128:(S + s + 1) * 128]

        def dcw(s):
            return diag_sb[:, (2 * S + s) * 128:(2 * S + s + 1) * 128]

        ident = diag_sb[:, 3 * S * 128:(3 * S + 1) * 128]

        a_view = [
            a_dram[s].ap().rearrange("(c p ft) -> c p ft", c=NCHUNK, p=128)
            for s in range(S)
        ]

        for _rep in range(repeat):
            # rnn2 input u2, layout [e_lo, m, l] with l = b_loc*2 + e_hi
            rnn2buf = r2p.tile([128, M, L2], FP32, tag="rnn2buf",
                               name="rnn2buf")
            for c in range(NCHUNK):
                xt = []
                for s, T in enumerate(SCALES):
                    x = xp[s].tile([128, F, T], FP32, tag=f"x{s}",
                                   name=f"x{s}")
                    nc.sync.dma_start(
                        out=x[:].rearrange("p f t -> p (f t)"),
                        in_=a_view[s][c])
                    xt.append(x)

                # interleave the three scales' recurrences step by step
                h_cur = [None] * S
                hfin = []
                for s in range(S):
                    hfin.append(hfp.tile([128, F], FP32, tag=f"hf{s}",
                                         name=f"hf{s}"))
                for t in range(max(SCALES)):
                    for s, T in enumerate(SCALES):
                        if t >= T:
                            continue
                        ps = ps1.tile([128, F], FP32, tag="ps", name="ps")
                        if t == 0:
                            nc.tensor.matmul(ps[:], dwih(s), xt[s][:, :, t],
                                             start=True, stop=True)
                        else:
                            nc.tensor.matmul(ps[:], dwih(s), xt[s][:, :, t],
                                             start=True, stop=False)
                            nc.tensor.matmul(ps[:], dwhh(s), h_cur[s][:],
                                             start=False, stop=True)
                        hn = hfin[s] if t == T - 1 else hp.tile(
                            [128, F], FP32, tag=f"h{s}", name=f"h{s}")
                        nc.scalar.activation(hn[:], ps[:], AF.Tanh,
                                             bias=bias_sb[:, s:s + 1])
                        h_cur[s] = hn

                # conv over scales + fold rnn2 input affine:
                #   u2 = wih2*(sum_s cw_s*h_s + cb) + bih2 + bhh2
                pc = psc.tile([128, F], FP32, tag="pc", name="pc")
                nc.tensor.matmul(pc[:], dcw(0), hfin[0][:],
                                 start=True, stop=False)
                nc.tensor.matmul(pc[:], dcw(1), hfin[1][:],
                                 start=False, stop=False)
                nc.tensor.matmul(pc[:], dcw(2), hfin[2][:],
                                 start=False, stop=True)
                cv = cvp.tile([128, F], FP32, tag="cv", name="cv")
                nc.scalar.activation(cv[:], pc[:], AF.Identity,
                                     bias=bias_sb[:, 3:4], scale=wih2)

                # transpose each 128x128 block; scatter into rnn2buf
                for j in range(4):
                    m_lo, e_hi = j // 2, j % 2
                    pt = pst.tile([128, 128], FP32, tag="pt", name="pt")
                    nc.tensor.transpose(pt[:], cv[:, j * 128:(j + 1) * 128],
                                        ident)
                    src = pt[:].rearrange("p (b v) -> p v b", b=4)
                    dst = rnn2buf[:, m_lo::2, 8 * c + e_hi:8 * c + 8:2]
                    nc.vector.tensor_copy(dst, src)

            # ---- rnn2 over m (truncated to last K steps) ----
            feat = smp.tile([128, L2], FP32, tag="feat", name="feat")
            h2 = None
            for m in range(M - K, M):
                last = m == M - 1
                dst = feat if last else smp.tile([128, L2], FP32, tag="h2",
                                                 name="h2")
                if h2 is None:
                    nc.scalar.activation(dst[:], rnn2buf[:, m, :], AF.Tanh)
                else:
                    st = smp.tile([128, L2], FP32, tag="st", name="st")
                    nc.vector.scalar_tensor_tensor(
                        st[:], h2[:], whh2, rnn2buf[:, m, :],
                        op0=ALU.mult, op1=ALU.add)
                    nc.scalar.activation(dst[:], st[:], AF.Tanh)
                h2 = dst

            # ---- BatchNorm stats (partial) ----
            featsq = smp.tile([128, L2], FP32, tag="fsq", name="fsq")
            nc.vector.tensor_tensor(featsq[:], feat[:], feat[:], ALU.mult)
            stats = smp.tile([128, 4], FP32, tag="stats", name="stats")
            fv = feat[:].rearrange("p (b eh) -> p eh b", b=BLOC)
            fsv = featsq[:].rearrange("p (b eh) -> p eh b", b=BLOC)
            nc.vector.tensor_reduce(stats[:, 0:2], fv,
                                    axis=mybir.AxisListType.X, op=ALU.add)
            nc.vector.tensor_reduce(stats[:, 2:4], fsv,
                                    axis=mybir.AxisListType.X, op=ALU.add)

        bin_ = dram.tile([128, 4], FP32, tag="bin")
        bout = dram.tile([128, 4], FP32, tag="bout")
        nc.gpsimd.dma_start(bin_[:], stats[:])
        nc.gpsimd.collective_compute(
            "AllReduce", ALU.add,
            replica_groups=[list(range(N_CORES))],
            ins=[bin_.opt()], outs=[bout.opt()])
        stg = smp.tile([128, 4], FP32, tag="stg")
        nc.gpsimd.dma_start(stg[:], bout[:])

        # mean/var/scale/shift (all (128,2): per (e_lo, e_hi))
        mean = smp.tile([128, 2], FP32, tag="mean")
        nc.vector.tensor_scalar(mean[:], stg[:, 0:2], 1.0 / B, None, ALU.mult)
        ex2 = smp.tile([128, 2], FP32, tag="ex2")
        nc.vector.tensor_scalar(ex2[:], stg[:, 2:4], 1.0 / B, None, ALU.mult)
        var = smp.tile([128, 2], FP32, tag="var")
        nc.vector.tensor_tensor(var[:], mean[:], mean[:], ALU.mult)
        nc.vector.tensor_tensor(var[:], ex2[:], var[:], ALU.subtract)
        lnv = smp.tile([128, 2], FP32, tag="lnv")
        nc.scalar.activation(lnv[:], var[:], AF.Ln, bias=bias_sb[:, 4:5])
        istd = smp.tile([128, 2], FP32, tag="istd")
        nc.scalar.activation(istd[:], lnv[:], AF.Exp, scale=-0.5)
        scl = smp.tile([128, 2], FP32, tag="scl")
        nc.vector.tensor_tensor(scl[:], istd[:], gb_sb[:, 0:2], ALU.mult)
        shf = smp.tile([128, 2], FP32, tag="shf")
        nc.vector.tensor_tensor(shf[:], mean[:], scl[:], ALU.mult)
        nc.vector.tensor_tensor(shf[:], gb_sb[:, 2:4], shf[:], ALU.subtract)

        # normalize + relu
        r = smp.tile([128, L2], FP32, tag="r")
        f3 = feat[:].rearrange("p (b eh) -> p b eh", b=BLOC)
        r3 = r[:].rearrange("p (b eh) -> p b eh", b=BLOC)
        for eh in range(2):
            nc.vector.tensor_scalar(
                r3[:, :, eh], f3[:, :, eh],
                scl[:, eh:eh + 1], shf[:, eh:eh + 1],
                op0=ALU.mult, op1=ALU.add)
        nc.vector.tensor_scalar_max(r[:], r[:], 0.0)

        # FC: logits^T (C, BLOC) = sum_eh Wpack_eh.T @ r[:, :, eh]
        pl = psf.tile([C, BLOC], FP32, tag="pl")
        nc.tensor.matmul(pl[:], wpack_sb[:, 0:C], r3[:, :, 0],
                         start=True, stop=False)
        nc.tensor.matmul(pl[:], wpack_sb[:, C:2 * C], r3[:, :, 1],
                         start=False, stop=True)
        lt = smp.tile([C, BLOC], FP32, tag="lt")
        nc.vector.tensor_scalar(lt[:], pl[:], fnnb_sb[:, 0:1], None, ALU.add)

        # transpose to (BLOC, C) and softmax along free dim
        pt2 = psf.tile([BLOC, C], FP32, tag="pt2")
        nc.tensor.transpose(pt2[:], lt[:], ident[0:C, 0:C])
        nmax = smp.tile([BLOC, 1], FP32, tag="nmax")
        nc.vector.tensor_reduce(nmax[:], pt2[:], axis=mybir.AxisListType.X,
                                op=ALU.max, negate=True)
        esb = smp.tile([BLOC, C], FP32, tag="esb")
        nc.scalar.activation(esb[:], pt2[:], AF.Exp, bias=nmax[:, 0:1])
        ssum = smp.tile([BLOC, 1], FP32, tag="ssum")
        nc.vector.tensor_reduce(ssum[:], esb[:], axis=mybir.AxisListType.X,
                                op=ALU.add)
        rin = smp.tile([BLOC, 1], FP32, tag="rin")
        nc.vector.reciprocal(rin[:], ssum[:])
        osb = smp.tile([BLOC, C], FP32, tag="osb")
        nc.vector.tensor_scalar(osb[:], esb[:], rin[:, 0:1], None, ALU.mult)
        nc.sync.dma_start(out=out_dram[:], in_=osb[:])

    nc.compile()
    return nc


def kernel(a0, a1, a2, rnn1_wih, rnn1_whh, rnn1_bih, rnn1_bhh,
           conv_w, conv_b, rnn2_wih, rnn2_whh, rnn2_bih, rnn2_bhh,
           norm_gamma, norm_beta, fnn_w, fnn_b, _bench=None):
    params = {
        "wih": [float(rnn1_wih[s]) for s in range(S)],
        "whh": [float(rnn1_whh[s]) for s in range(S)],
        "bb": [float(rnn1_bih[s]) + float(rnn1_bhh[s]) for s in range(S)],
        "cw": [float(conv_w[s]) for s in range(S)],
        "cb": float(conv_b[0]),
        "wih2": float(rnn2_wih[0]),
        "whh2": float(rnn2_whh[0]),
        "bb2": float(rnn2_bih[0]) + float(rnn2_bhh[0]),
        "gamma": np.asarray(norm_gamma, np.float32),
        "beta": np.asarray(norm_beta, np.float32),
        "fnn_w": np.asarray(fnn_w, np.float32),
        "fnn_b": np.asarray(fnn_b, np.float32),
    }
    nc = _build(params)

    flat = [np.ascontiguousarray(np.asarray(a, np.float32)).reshape(-1)
            for a in (a0, a1, a2)]
    in_maps = []
    for k in range(N_CORES):
        m = {}
        for i, T in enumerate(SCALES):
            sz = N8 * T
            m[f"a{i}"] = flat[i][k * sz:(k + 1) * sz]
        in_maps.append(m)

    kw = dict(_bench) if _bench else {}
    res = run_bass_kernel_spmd(nc, in_maps, core_ids=list(range(N_CORES)),
                               **kw)
    out = np.concatenate([res.results[k]["out"] for k in range(N_CORES)],
                         axis=0)
    if _bench is not None:
        kernel.last_result = res
    return out
